# revision 2
# baseline (speedup 1.0000x reference)
"""StyleGAN2-style modulated 3x3 conv on 8 TRN2 cores - fp8 DoubleRow variant.

Same folded math + width-Winograd F(4,3) structure as kernel.py, but the conv
matmuls run in fp8e4 DoubleRow mode (2 k-tiles of 128 per instruction, 0.5
cyc/col) with full residual compensation to hold accuracy:

    conv = V8*W8 + V8*R8 + S8*W8        (all DoubleRow, one PSUM bank/g)
      W8 = fp8(alpha_g*U),  R8 = fp8(alpha_g*U - W8)     (host precomp)
      V8 = fp8(V),          S8 = fp8(V - V8)             (on device)

alpha_g = [4,4,4,16,16,1] normalizes the G-matrix row magnitudes so the
residuals R stay out of fp8 subnormal range; 1/alpha_g folds into the per-g
PSUM-evac scale (dscale_g = dscale/alpha_g).

Per half (34 rows): DVE builds the 24 fp16 B^T planes into a 12-buffer ring;
ACT casts each plane to V8, Pool (gpsimd) computes S8 = V - V8. Casts/subs are
drained 6-per-co-block inside the next conv half so the in-order ACT queue
never blocks PSUM evacs. V8/S8 are stored per (g,half) as [128, 4*544] tiles
so DoubleRow rhs APs pair adjacent ci-tiles at stride 544.

Output is stored fp16 (host upcasts) to halve the store DMA.
"""

import numpy as np
import ml_dtypes

B, CIN, COUT, K, LAT, H, W = 8, 512, 512, 3, 512, 64, 64
EPS = 1e-8
W_MUL_FC = LAT**-0.5
W_MUL_CONV = (2.0**0.5) * (CIN * K * K) ** -0.5

P = 128
CI_T = CIN // P  # 4 input-channel tiles
CO_T = COUT // P  # 4 output-channel tiles
LA_T = LAT // P  # 4 latent tiles
WP = 72  # padded row width the phases are cut from
HP = H + 2  # padded height (66)
NJ = W // 4  # 16 winograd tiles per row
JW = NJ + 1  # 17 j-entries per phase plane
PLC = HP * JW  # 1122 cols per phase plane
XCOLS = 4 * PLC  # 4488
HR = 34  # rows per V half
VHC = HR * NJ  # 544 cols per V half-plane
NMAX = 512
ALPHA = [4.0, 4.0, 4.0, 16.0, 16.0, 1.0]  # per-g weight prescale
NVF = 10  # fp16 B^T plane ring buffers

_COMPILED = {}


def _build_nc(reps=1):
    import concourse.bass as bass
    import concourse.mybir as mybir
    from concourse import bacc
    from concourse.tile import TileContext

    fp32 = mybir.dt.float32
    fp16 = mybir.dt.float16
    fp8 = mybir.dt.float8e4
    AF = mybir.ActivationFunctionType
    ALU = mybir.AluOpType
    DR = mybir.MatmulPerfMode.DoubleRow

    nc = bacc.Bacc("TRN2", target_bir_lowering=False, debug=False)

    xp_d = nc.dram_tensor("xp", [CI_T, P, XCOLS], fp16, kind="ExternalInput")
    lat_d = nc.dram_tensor("lat", [LAT], fp16, kind="ExternalInput")
    # DoubleRow weight pairs: [ci-pair, P, ((co*3+kh)*6+g)*2*P + kt*P + m]
    wt_d = nc.dram_tensor("wt8", [2, P, CO_T * 18 * 2 * P], fp8, kind="ExternalInput")
    rt_d = nc.dram_tensor("rt8", [2, P, CO_T * 18 * 2 * P], fp8, kind="ExternalInput")
    fct_d = nc.dram_tensor("fct", [LA_T, P, CIN], fp16, kind="ExternalInput")
    ssq_d = nc.dram_tensor("ssq", [CI_T, P, COUT], fp8, kind="ExternalInput")
    fcb_d = nc.dram_tensor("fcb", [P, CI_T], fp32, kind="ExternalInput")
    cb_d = nc.dram_tensor("cbias", [P, CO_T], fp32, kind="ExternalInput")
    out_d = nc.dram_tensor("out", [COUT, H, W], fp16, kind="ExternalOutput")

    inv_wmc2 = 1.0 / (W_MUL_CONV * W_MUL_CONV)

    with TileContext(nc) as tc, tc.tile_pool(name="persist", bufs=1) as persist:
        def tile0(shape, dtype, name):
            return persist.tile(shape, dtype, tag=name, name=name)

        fct = [tile0([P, CIN], fp16, f"fct{i}") for i in range(LA_T)]
        fcb = tile0([P, CI_T], fp32, "fcb")
        cb = tile0([P, CO_T], fp32, "cb")
        ssq = [tile0([P, COUT], fp8, f"ssq{i}") for i in range(CI_T)]
        wsb = [tile0([P, CO_T * 18 * 2 * P], fp8, f"wsb{i}") for i in range(2)]
        rsb = [tile0([P, CO_T * 18 * 2 * P], fp8, f"rsb{i}") for i in range(2)]

        for l in range(LA_T):
            nc.sync.dma_start(fct[l][:], fct_d[l])
        nc.sync.dma_start(fcb[:], fcb_d[:])
        nc.sync.dma_start(cb[:], cb_d[:])
        for ci in range(CI_T):
            nc.sync.dma_start(ssq[ci][:], ssq_d[ci])

        with (
            tc.tile_pool(name="xpool", bufs=1) as xpool,
            tc.tile_pool(name="vf", bufs=NVF) as vfpool,
            tc.tile_pool(name="v8pool", bufs=1) as v8pool,
            tc.tile_pool(name="s8pool", bufs=1) as s8pool,
            tc.tile_pool(name="vtmp", bufs=1) as vtpool,
            tc.tile_pool(name="mpool", bufs=1) as mpool,
            tc.tile_pool(name="spool", bufs=2) as spool,
            tc.tile_pool(name="pconv", bufs=8, space="PSUM") as pconv,
            tc.tile_pool(name="ypool", bufs=1) as ypool,
            tc.tile_pool(name="ycpool", bufs=2) as ycpool,
            tc.tile_pool(name="ytout", bufs=2) as ytpool,
            tc.tile_pool(name="dtmp", bufs=1) as dpool,
        ):
            state = {}

            def emit_inputs_style_demod(rep):
                latsb = spool.tile([P, LA_T], fp16, tag="latsb", name=f"latsb_{rep}")
                nc.sync.dma_start(latsb[:], lat_d[:].rearrange("(l p) -> p l", p=P))
                xmod = [
                    xpool.tile([P, XCOLS], fp16, tag=f"xmod{i}", name=f"xmod{i}_{rep}")
                    for i in range(CI_T)
                ]
                for ci in range(CI_T):
                    nc.sync.dma_start(xmod[ci][:], xp_d[ci])
                style = [
                    spool.tile([P, 1], fp32, tag=f"style{i}", name=f"style{i}_{rep}")
                    for i in range(CI_T)
                ]
                style2 = [
                    spool.tile([P, 1], fp16, tag=f"style2_{i}", name=f"style2_{i}_{rep}")
                    for i in range(CI_T)
                ]
                # dscale variants per g (1/alpha_g folded in)
                dscale = [
                    [
                        spool.tile(
                            [P, 1], fp32, tag=f"dsc{co}_{g}", name=f"dsc{co}_{g}_{rep}"
                        )
                        for g in range(6)
                    ]
                    for co in range(CO_T)
                ]
                for ci in range(CI_T):
                    ps = pconv.tile(
                        [P, NMAX], fp32, tag="ps_conv", name=f"ps_st{ci}_{rep}"
                    )[:, :1]
                    for l in range(LA_T):
                        nc.tensor.matmul(
                            ps[:],
                            lhsT=fct[l][:, ci * P : (ci + 1) * P],
                            rhs=latsb[:, l : l + 1],
                            start=(l == 0),
                            stop=(l == LA_T - 1),
                        )
                    nc.scalar.activation(
                        style[ci][:], ps[:], AF.Identity,
                        bias=fcb[:, ci : ci + 1], scale=W_MUL_FC,
                    )
                    nc.scalar.activation(
                        style2[ci][:], ps[:], AF.Square,
                        bias=fcb[:, ci : ci + 1], scale=W_MUL_FC,
                    )
                    nc.vector.tensor_scalar_mul(xmod[ci][:], xmod[ci][:], style[ci][:])

                for co in range(CO_T):
                    ps = pconv.tile(
                        [P, NMAX], fp32, tag="ps_conv", name=f"ps_d{co}_{rep}"
                    )[:, :1]
                    for ci in range(CI_T):
                        nc.tensor.matmul(
                            ps[:],
                            lhsT=ssq[ci][:, co * P : (co + 1) * P],
                            rhs=style2[ci][:],
                            start=(ci == 0),
                            stop=(ci == CI_T - 1),
                        )
                    sarg = dpool.tile([P, 1], fp32, tag="sarg", name=f"sarg{co}_{rep}")
                    sq = dpool.tile([P, 1], fp32, tag="sq", name=f"sq{co}_{rep}")
                    y0 = dpool.tile([P, 1], fp32, tag="y0", name=f"y0_{co}_{rep}")
                    u = dpool.tile([P, 1], fp32, tag="u", name=f"u{co}_{rep}")
                    v = dpool.tile([P, 1], fp32, tag="v", name=f"v{co}_{rep}")
                    nc.scalar.activation(
                        sarg[:], ps[:], AF.Identity, bias=0.0, scale=inv_wmc2
                    )
                    nc.scalar.activation(sq[:], ps[:], AF.Sqrt, bias=0.0, scale=inv_wmc2)
                    nc.vector.reciprocal(y0[:], sq[:])
                    nc.vector.tensor_mul(u[:], y0[:], y0[:])
                    nc.vector.tensor_mul(v[:], u[:], sarg[:])
                    nc.vector.tensor_scalar(
                        v[:], v[:], -0.5, 1.5, op0=ALU.mult, op1=ALU.add
                    )
                    nc.vector.tensor_mul(v[:], y0[:], v[:])
                    for g in range(6):
                        nc.vector.tensor_scalar_mul(
                            dscale[co][g][:], v[:], 1.0 / ALPHA[g]
                        )
                state[rep] = {"xmod": xmod, "dscale": dscale}

            def emit_vbuild_half(rep, h):
                """B^T width combos for rows 32h..32h+33 -> fp16 ring, and
                register the pending V8-cast/S8-sub work (drained later)."""
                xmod = state[rep]["xmod"]
                v8 = [
                    v8pool.tile([P, 4 * VHC], fp8, tag=f"v8_{h}_{g}",
                                name=f"v8_{h}_{g}_{rep}")
                    for g in range(6)
                ]
                s8 = [
                    s8pool.tile([P, 4 * VHC], fp8, tag=f"s8_{h}_{g}",
                                name=f"s8_{h}_{g}_{rep}")
                    for g in range(6)
                ]
                state[rep][f"V8_{h}"] = v8
                state[rep][f"S8_{h}"] = s8
                vf = {}  # (g, ci) -> fp16 plane tile
                r0 = 32 * h
                for ci in range(CI_T):
                    def Pl(a):
                        return xmod[ci][
                            :, a * PLC : (a + 1) * PLC
                        ].rearrange("p (h j) -> p h j", j=JW)[:, r0 : r0 + HR, :]

                    d0 = Pl(0)[:, :, 0:NJ]
                    d1 = Pl(1)[:, :, 0:NJ]
                    d2 = Pl(2)[:, :, 0:NJ]
                    d3 = Pl(3)[:, :, 0:NJ]
                    d4 = Pl(0)[:, :, 1 : NJ + 1]
                    d5 = Pl(1)[:, :, 1 : NJ + 1]

                    for g in range(6):
                        vf[(g, ci)] = vfpool.tile(
                            [P, VHC], fp16, tag="vf", name=f"vf{h}_{g}_{ci}_{rep}"
                        )

                    def vt(g):
                        return vf[(g, ci)][:].rearrange("p (h j) -> p h j", j=NJ)

                    def tmp(nm):
                        t = vtpool.tile(
                            [P, VHC], fp16, tag=nm, name=f"{nm}{h}_{ci}_{rep}"
                        )
                        return t[:].rearrange("p (h j) -> p h j", j=NJ)

                    a2 = tmp("ca")
                    t1 = tmp("cb")
                    nc.vector.tensor_sub(a2, d4, d2)
                    nc.vector.tensor_sub(t1, d0, d2)
                    nc.vector.tensor_scalar_mul(t1, t1, 4.0)
                    nc.vector.tensor_add(vt(0), t1, a2)
                    b2 = tmp("cc")
                    b1 = tmp("cb")
                    nc.vector.tensor_add(b2, d1, d2)
                    nc.vector.tensor_scalar_mul(b2, b2, 4.0)
                    nc.vector.tensor_add(b1, d3, d4)
                    nc.vector.tensor_sub(vt(1), b1, b2)
                    c2 = tmp("cb")
                    c1x = tmp("cc")
                    nc.vector.tensor_sub(c2, d1, d2)
                    nc.vector.tensor_scalar_mul(c2, c2, 4.0)
                    nc.vector.tensor_sub(c1x, d4, d3)
                    nc.vector.tensor_add(vt(2), c2, c1x)
                    bb = tmp("cb")
                    nc.vector.tensor_sub(bb, d3, d1)
                    nc.vector.tensor_scalar_mul(bb, bb, 2.0)
                    nc.vector.tensor_add(vt(3), a2, bb)
                    nc.vector.tensor_sub(vt(4), a2, bb)
                    e2 = tmp("cc")
                    nc.vector.tensor_sub(e2, d5, d3)
                    nc.vector.scalar_tensor_tensor(
                        vt(5), bb, -2.0, e2, op0=ALU.mult, op1=ALU.add
                    )
                # pending cast/sub work, plane order (ci-major to match ring age)
                pend = []
                for ci in range(CI_T):
                    for g in range(6):
                        pend.append((g, ci, vf[(g, ci)]))
                state[rep][f"pend_{h}"] = pend

            def drain_pending(rep, h, k):
                """Emit k pending (V8 cast on ACT, S8 sub on Pool) pairs."""
                pend = state[rep][f"pend_{h}"]
                v8 = state[rep][f"V8_{h}"]
                s8 = state[rep][f"S8_{h}"]
                for _ in range(min(k, len(pend))):
                    g, ci, vf = pend.pop(0)
                    sl = slice(ci * VHC, (ci + 1) * VHC)
                    nc.scalar.activation(
                        v8[g][:, sl], vf[:], AF.Identity, bias=0.0, scale=1.0
                    )
                    nc.gpsimd.tensor_sub(s8[g][:, sl], vf[:], v8[g][:, sl])

            prev_blk = [None]

            def flush_block(pb):
                pco, pbb, pyc, pyt = pb
                pytv = pyt[:].rearrange("p (r j t) -> p r j t", j=NJ, t=4)
                for t in range(4):
                    nc.scalar.activation(
                        pytv[:, :, :, t].rearrange("p r j -> p (r j)"),
                        pyc[t][:], AF.Prelu,
                        bias=cb[:, pco : pco + 1], scale=1.0, alpha=0.2,
                    )
                nc.sync.dma_start(
                    out_d[pco * P : (pco + 1) * P, 32 * pbb : 32 * pbb + 32, :],
                    pyt[:].rearrange("p (r w) -> p r w", w=W),
                )

            def emit_conv_half(rep, b, drain=None):
                """All 4 co-tiles for row-block b; drains pending casts/subs
                for `drain`=(rep2, h2) 6 per co-block."""
                dscale = state[rep]["dscale"]
                v8 = state[rep][f"V8_{b}"]
                s8 = state[rep][f"S8_{b}"]
                for co in range(CO_T):
                    msb = []
                    for g in range(6):
                        ps = pconv.tile(
                            [P, NMAX], fp32, tag="ps_conv", name=f"pc{co}_{b}_{g}_{rep}"
                        )
                        v8v = v8[g][:].rearrange("p (c x) -> p c x", c=CI_T)
                        s8v = s8[g][:].rearrange("p (c x) -> p c x", c=CI_T)
                        idx = 0
                        for wl, rh in ((wsb, v8v), (rsb, v8v), (wsb, s8v)):
                            for q in range(2):
                                for kh in range(3):
                                    off = ((co * 3 + kh) * 6 + g) * 2 * P
                                    nc.tensor.matmul(
                                        ps[:],
                                        lhsT=wl[q][:, off : off + 2 * P].rearrange(
                                            "p (k m) -> p k m", k=2
                                        ),
                                        rhs=rh[
                                            :, 2 * q : 2 * q + 2,
                                            kh * NJ : kh * NJ + NMAX,
                                        ],
                                        start=(idx == 0),
                                        stop=(idx == 17),
                                        perf_mode=DR,
                                    )
                                    idx += 1
                        m = mpool.tile(
                            [P, NMAX], fp16, tag=f"m{g}", name=f"m{g}_{co}_{b}_{rep}"
                        )
                        nc.scalar.activation(
                            m[:], ps[:], AF.Identity, bias=0.0,
                            scale=dscale[co][g][:],
                        )
                        msb.append(m)

                    if prev_blk[0] is not None:
                        flush_block(prev_blk[0])
                        prev_blk[0] = None
                    if drain is not None:
                        drain_pending(drain[0], drain[1], 6)

                    def ytile(nm):
                        return ypool.tile(
                            [P, NMAX], fp16, tag=nm, name=f"{nm}_{co}_{b}_{rep}"
                        )

                    tp = ytile("tp")
                    tq = ytile("tq")
                    tr = ytile("tr")
                    ts = ytile("ts")
                    yc = [
                        ycpool.tile(
                            [P, NMAX], fp16, tag=f"yc{t}", name=f"yc{t}_{co}_{b}_{rep}"
                        )
                        for t in range(4)
                    ]
                    yt = ytpool.tile(
                        [P, 4 * NMAX], fp16, tag="yt", name=f"yt_{co}_{b}_{rep}"
                    )

                    nc.vector.tensor_add(tp[:], msb[1][:], msb[2][:])
                    nc.vector.tensor_sub(tq[:], msb[1][:], msb[2][:])
                    nc.vector.tensor_add(tr[:], msb[3][:], msb[4][:])
                    nc.vector.tensor_sub(ts[:], msb[3][:], msb[4][:])
                    nc.vector.tensor_add(yc[0][:], tp[:], msb[0][:])
                    nc.vector.tensor_add(yc[0][:], yc[0][:], tr[:])
                    nc.vector.tensor_scalar_mul(ts[:], ts[:], 2.0)
                    nc.vector.tensor_add(yc[1][:], tq[:], ts[:])
                    nc.vector.tensor_scalar_mul(tr[:], tr[:], 4.0)
                    nc.vector.tensor_add(yc[2][:], tp[:], tr[:])
                    nc.vector.tensor_scalar_mul(ts[:], ts[:], 4.0)
                    nc.vector.tensor_add(ts[:], ts[:], msb[5][:])
                    nc.vector.tensor_add(yc[3][:], tq[:], ts[:])
                    prev_blk[0] = (co, b, yc, yt)

            # --- software-pipelined rep loop ---
            for q in range(2):
                nc.sync.dma_start(wsb[q][:, : 18 * 2 * P], wt_d[q, :, : 18 * 2 * P])
            emit_inputs_style_demod(0)
            for co in range(1, CO_T):
                for q in range(2):
                    s = co * 18 * 2 * P
                    nc.sync.dma_start(
                        wsb[q][:, s : s + 18 * 2 * P], wt_d[q, :, s : s + 18 * 2 * P]
                    )
            for q in range(2):
                nc.sync.dma_start(rsb[q][:], rt_d[q])
            emit_vbuild_half(0, 0)
            drain_pending(0, 0, 24)
            for rep in range(reps):
                emit_vbuild_half(rep, 1)
                emit_conv_half(rep, 0, drain=(rep, 1))
                if rep + 1 < reps:
                    emit_inputs_style_demod(rep + 1)
                    emit_vbuild_half(rep + 1, 0)
                    emit_conv_half(rep, 1, drain=(rep + 1, 0))
                else:
                    emit_conv_half(rep, 1)
            if prev_blk[0] is not None:
                flush_block(prev_blk[0])
                prev_blk[0] = None

    nc.compile()
    return nc


def _get_compiled(reps=1):
    if reps not in _COMPILED:
        _COMPILED[reps] = _build_nc(reps)
    return _COMPILED[reps]


def _prep_inputs(x, latent, weight, bias, fc_weight, fc_bias):
    fp16 = np.float16
    f8 = ml_dtypes.float8_e4m3fn
    xp72 = np.pad(x, ((0, 0), (0, 0), (1, 1), (1, WP - W - 1)))
    ph = np.stack([xp72[:, :, :, a::4][:, :, :, :JW] for a in range(4)], axis=2)
    xp = np.ascontiguousarray(
        ph.reshape(B, CIN, XCOLS).reshape(B, CI_T, P, XCOLS)
    ).astype(fp16)

    w6 = weight.astype(np.float64).reshape(CO_T, P, CI_T, P, 3, 3)
    Gm = np.array(
        [
            [1 / 4, 0, 0],
            [-1 / 6, -1 / 6, -1 / 6],
            [-1 / 6, 1 / 6, -1 / 6],
            [1 / 24, 1 / 12, 1 / 6],
            [1 / 24, -1 / 12, 1 / 6],
            [0, 0, 1],
        ]
    )
    U = np.einsum("gw,omipkw->omipkg", Gm, w6)  # [co, m, ci, p, kh, g]
    U = U * np.asarray(ALPHA)[None, None, None, None, None, :]
    U8 = U.astype(np.float32).astype(f8)
    R = U - U8.astype(np.float64)
    R8 = R.astype(np.float32).astype(f8)

    # DoubleRow lhsT layout: wt8[q, p, (((co*3+kh)*6+g)*2 + kt)*P + m]
    #   = U8[co, m, ci=2q+kt, p, kh, g]
    def pack(Uq):
        arr = Uq.astype(np.float32).reshape(CO_T, P, 2, 2, P, 3, 6)
        # [co, m, q, kt, p, kh, g] -> [q, p, co, kh, g, kt, m]
        arr = arr.transpose(2, 4, 0, 5, 6, 3, 1)
        return np.ascontiguousarray(arr).reshape(2, P, CO_T * 18 * 2 * P).astype(f8)

    wt8 = pack(U8)
    rt8 = pack(R8)

    fct = np.ascontiguousarray(fc_weight.T).reshape(LA_T, P, CIN).astype(fp16)
    ssq = np.ascontiguousarray(
        (weight.astype(np.float64) ** 2).sum(axis=(2, 3)).T
    ).reshape(CI_T, P, COUT).astype(f8)
    fcb = np.ascontiguousarray(fc_bias.reshape(CI_T, P).T).astype(np.float32)
    cb = np.ascontiguousarray(bias.reshape(CO_T, P).T).astype(np.float32)
    lat = np.ascontiguousarray(latent).astype(fp16)

    in_maps = []
    for b in range(B):
        in_maps.append(
            {
                "xp": xp[b],
                "lat": lat[b],
                "wt8": wt8,
                "rt8": rt8,
                "fct": fct,
                "ssq": ssq,
                "fcb": fcb,
                "cbias": cb,
            }
        )
    return in_maps


def kernel(x, latent, weight, bias, fc_weight, fc_bias):
    from concourse.bass_utils import run_bass_kernel_spmd

    x = np.asarray(x, np.float32)
    latent = np.asarray(latent, np.float32)
    weight = np.asarray(weight, np.float32)
    bias = np.asarray(bias, np.float32)
    fc_weight = np.asarray(fc_weight, np.float32)
    fc_bias = np.asarray(fc_bias, np.float32)

    nc = _get_compiled()
    in_maps = _prep_inputs(x, latent, weight, bias, fc_weight, fc_bias)
    res = run_bass_kernel_spmd(nc, in_maps, core_ids=list(range(B)))
    out = np.stack([res.results[b]["out"] for b in range(B)], axis=0)
    return out.astype(np.float32)
